# revision 6
# baseline (speedup 1.0000x reference)
"""CategorySpecificLinear TRN2 kernel.

out[b] = x[b] @ W[cat_ids[b]] + bias[cat_ids[b]]
  x: [64, 512, 1024] f32, W: [32, 1024, 4096] f32, b: [32, 4096] f32
  -> out [64, 512, 4096] f32

Strategy: data-parallel over batch — 8 batches per core on 8 NeuronCores.
The category gather, fp16 conversion, and x transpose are done on the host
(cat_ids are known at launch), so each core receives its 8 per-batch weight
matrices directly; no on-device indexing is needed. Matmuls run in fp16 with
fp32 PSUM accumulation: same PE throughput as bf16 on TRN2 (1 cycle/row) but
~8x better accuracy (~3e-4 rel), and 4x faster than native fp32 (4 cycles/row).
fp8 was measured and rejected: DoubleRow pair-instrs run at the same 225ns as
fp16 matmuls (2x FLOPs only), and any accuracy-preserving split needs 2 fp8
slots per logical k -> ties fp16; single-pass e4m3 rel err 0.038 > 2e-2 gate.

Per core: 2048 matmuls of [128k,128m]@[128k,512n] at the warm issue-rate
floor (~216 ns each => 442 us PE floor); ~14 us of NEFF preamble/teardown is
fixed. Overhead trims vs the 466-488 us baseline:
 - first batch's w loads ride the (initially idle) scalar HWDGE queue while
   x rides sync: their descriptors issue in parallel at startup (descriptor
   issue is ~640 ns each, serial per queue)
 - first batch's x/w loads split k-tile-fine so matmul 0 starts ~2 us earlier
 - ~10 garbage warmup matmuls at t~6 us (during the initial DMA window) ramp
   the PE p-state so the real stream opens at 2.4 GHz, not 1.2 GHz
 - outputs stored fp16 (halves output-queue bytes; host upcasts), and the
   last n-tile's writes alternate scalar/sync queues to halve the end drain
 - all 8 PSUM banks cycle; bias rows ride the gpsimd DMA queue with the
   6.2 us gpsimd partition-broadcast hidden under the previous batch
"""
import numpy as np

B_TOTAL = 64
N_CORES = 8
B = B_TOTAL // N_CORES  # batches per core
S = 512    # seq
K = 1024   # input_dim
H = 4096   # hidden_dim
P = 128
KT = K // P   # 8 k-tiles
MT = S // P   # 4 m-tiles
NW = 512      # hidden tile width (one fp32 PSUM bank)
NT = H // NW  # 8 n-tiles

_NC = None


def _build_nc():
    global _NC
    if _NC is not None:
        return _NC

    import concourse.mybir as mybir
    import concourse.tile as tile
    from concourse import bacc

    f16 = mybir.dt.float16
    f32 = mybir.dt.float32

    nc = bacc.Bacc("TRN2", target_bir_lowering=False, debug=False, num_devices=N_CORES)
    xt = nc.dram_tensor("xt", [B, K, S], f16, kind="ExternalInput").ap()
    w = nc.dram_tensor("w", [B, K, H], f16, kind="ExternalInput").ap()
    bias = nc.dram_tensor("bias", [B, H], f32, kind="ExternalInput").ap()
    out = nc.dram_tensor("out", [B, S, H], f16, kind="ExternalOutput").ap()

    def load_x(xt_sb, b_i, k0, k1):
        nc.sync.dma_start(
            xt_sb[:, k0:k1, :],
            xt[b_i, k0 * P : k1 * P, :].rearrange("(ko p) s -> p ko s", p=P),
        )

    def load_w(w_sb, b_i, n_i, k0, k1, eng=None):
        (eng or nc.sync).dma_start(
            w_sb[:, k0:k1, :],
            w[b_i, k0 * P : k1 * P, n_i * NW : (n_i + 1) * NW].rearrange(
                "(ko p) n -> p ko n", p=P
            ),
        )

    with tile.TileContext(nc) as tc:
        with (
            tc.tile_pool(name="xtp", bufs=2) as xtp,
            tc.tile_pool(name="wp", bufs=6) as wp,
            tc.tile_pool(name="bp", bufs=3) as bp,
            tc.tile_pool(name="op", bufs=8) as op,
            tc.tile_pool(name="wu", bufs=1) as wu,
            tc.tile_pool(name="ps", bufs=8, space="PSUM") as ps,
        ):
            # PE p-state warmup: ~10 matmuls on a zeroed tile, no DMA deps,
            # so they run during the initial input-DMA window and the real
            # stream starts at full clock. Result is never read.
            wsrc = wu.tile([P, NW], f16, tag="warm")
            nc.vector.memset(wsrc[:], 0.0)
            wps = ps.tile([P, NW], f32, tag="psum")
            for _ in range(10):
                nc.tensor.matmul(
                    wps[:], wsrc[:, 0:P], wsrc[:], start=True, stop=True
                )

            for b_i in range(B):
                # bias row rides the gpsimd DMA queue (never blocks x/w);
                # broadcast runs on gpsimd, hidden under the previous
                # batch's matmuls (bufs=3 gives prefetch slack)
                bias_row = bp.tile([1, H], f32, tag="bias_row")
                nc.gpsimd.dma_start(bias_row[:], bias[b_i][None, :])
                xt_sb = xtp.tile([P, KT, S], f16, tag="xt")
                w0_sb = wp.tile([P, KT, NW], f16, tag="w")
                if b_i == 0:
                    # k-tile-fine first chunks on parallel queues: x rides
                    # sync, w rides the (still-idle) scalar queue, so
                    # matmul 0 waits only on x[k0] || w[k0]
                    for k in range(2):
                        load_x(xt_sb, b_i, k, k + 1)
                        load_w(w0_sb, b_i, 0, k, k + 1, eng=nc.scalar)
                    for sp in range(1, 4):
                        load_x(xt_sb, b_i, sp * 2, sp * 2 + 2)
                        load_w(w0_sb, b_i, 0, sp * 2, sp * 2 + 2, eng=nc.scalar)
                else:
                    for sp in range(4):
                        load_x(xt_sb, b_i, sp * 2, sp * 2 + 2)
                        load_w(w0_sb, b_i, 0, sp * 2, sp * 2 + 2)
                bias_bc = bp.tile([P, H], f32, tag="bias_bc")
                nc.gpsimd.partition_broadcast(bias_bc[:], bias_row[:])
                for n_i in range(NT):
                    if n_i == 0:
                        w_sb = w0_sb
                    else:
                        w_sb = wp.tile([P, KT, NW], f16, tag="w")
                        for sp in range(2):
                            load_w(w_sb, b_i, n_i, sp * (KT // 2), (sp + 1) * (KT // 2))
                    for m_i in range(MT):
                        pt = ps.tile([P, NW], f32, tag="psum")
                        for k_i in range(KT):
                            nc.tensor.matmul(
                                pt[:],
                                xt_sb[:, k_i, m_i * P : (m_i + 1) * P],
                                w_sb[:, k_i, :],
                                start=(k_i == 0),
                                stop=(k_i == KT - 1),
                            )
                        ot = op.tile([P, NW], f16, tag="out")
                        nc.vector.tensor_add(
                            ot[:], pt[:], bias_bc[:, n_i * NW : (n_i + 1) * NW]
                        )
                        # outputs ride the scalar HWDGE queue so bursts don't
                        # head-of-line-block weight loads; the final n-tile
                        # alternates with the (by-then idle) sync queue to
                        # halve the end-of-kernel drain
                        eng = nc.scalar
                        if b_i == B - 1 and n_i == NT - 1 and m_i % 2 == 1:
                            eng = nc.sync
                        eng.dma_start(
                            out[b_i, m_i * P : (m_i + 1) * P, n_i * NW : (n_i + 1) * NW],
                            ot[:],
                        )
    nc.compile()
    _NC = nc
    return nc


def _prep_in_maps(x, cat_ids, W, b):
    W16 = W.astype(np.float16)                      # [32, K, H]
    Wg = W16[cat_ids]                               # [64, K, H]
    x16 = x.astype(np.float16)                      # [64, S, K]
    xt16 = np.ascontiguousarray(x16.transpose(0, 2, 1))  # [64, K, S]
    bg = b[cat_ids].astype(np.float32)              # [64, H]

    in_maps = []
    for c in range(N_CORES):
        sl = slice(B * c, B * (c + 1))
        in_maps.append(
            {
                "xt": np.ascontiguousarray(xt16[sl]),
                "w": np.ascontiguousarray(Wg[sl]),
                "bias": np.ascontiguousarray(bg[sl]),
            }
        )
    return in_maps


def kernel(x, cat_ids, W, b):
    from concourse.bass_utils import run_bass_kernel_spmd

    x = np.asarray(x, dtype=np.float32)
    cat_ids = np.asarray(cat_ids).astype(np.int64)
    W = np.asarray(W, dtype=np.float32)
    b = np.asarray(b, dtype=np.float32)

    nc = _build_nc()
    in_maps = _prep_in_maps(x, cat_ids, W, b)
    res = run_bass_kernel_spmd(nc, in_maps, core_ids=list(range(N_CORES)))
    out = np.concatenate([r["out"] for r in res.results], axis=0)
    return out.astype(np.float32, copy=False)
